# revision 43
# baseline (speedup 1.0000x reference)
"""Trainium2 Bass kernel for nn_ConceptLayer (B=2, S=512, E=256), 8 NeuronCores.

Math:
  s[b,i,:] = sum_{j<i} x[b,j,:] / (i-j)^2            (prefix matmul, W constant)
  y[b,i,c] = sum_{a,p} x[b,i,a] * s[b,i,p] * C[c,a,p]
  out      = LayerNorm(y + x) * gamma + beta          (eps=1e-3)

Sharding: contraction axis `a` split 8 ways (32 a's per core); each core streams
1/8 of the concept_map (bf16, 4MB) once into SBUF and keeps it resident.

v2 device algorithm per core:
  phase 1: SmT[p,t] = s^T via PE (x bf16 stationary, W^T bf16 moving) -> bf16
  phase 2: token-group pipeline (G groups of T/G tokens):
      for each a-pair: psum[t,2*E] = SmT^T @ [C_a0|C_a1]  (bf16 matmul, N=512)
      DVE: y[t] (+)= x[t,a] * psum-half   (scalar_tensor_tensor, per-partition
           scalar; final a writes bf16 y_fin directly)
      after each group: DMA y_fin -> ccin_g, AllToAll (bf16, overlapped with the
      next group's matmuls), local 8-way reduce, +x residual, LayerNorm, store.
Host gathers the 8 [128,E] outputs and reorders token slices.
"""
import os
import numpy as np

import concourse.bass as bass
import concourse.mybir as mybir
import concourse.tile as tile
from concourse.bass_utils import run_bass_kernel_spmd

# ----------------------------------------------------------------------------
# constants (hardcoded per problem spec)
B, S, E = 2, 512, 256
T = B * S                      # 1024 tokens
NCORES = 8
ASH = E // NCORES              # 32 contraction-a values per core
TCH = T // 128                 # 8 token chunks of 128
LN_EPS = 1e-3

G = int(os.environ.get("KG", "2"))          # token groups (collective staging)
TPG = TCH // G                 # t-chunks per group
RPG = 128 // G                 # output rows per (core, group)
SPG = T // G // NCORES         # tokens per (core, group) == RPG

F32 = mybir.dt.float32
BF16 = mybir.dt.bfloat16
MULT = mybir.AluOpType.mult
ADD = mybir.AluOpType.add
AF = mybir.ActivationFunctionType

MODE = os.environ.get("KMODE", "v3")

LAST_RESULTS = None            # BassKernelResults of the last run (for test.py)

_NC_CACHE = {}


def _install_ntff_hook():
    """antenv.axon_hooks is absent in this image; recreate it so
    run_bass_kernel_spmd(trace=True) can drive NTFF profiling via the
    libaxon_pjrt.so C ABI (same recipe as trn_agent_boot)."""
    import sys, types, ctypes, contextlib  # noqa: E401

    if "antenv.axon_hooks" in sys.modules:
        return
    so_path = "/opt/axon/libaxon_pjrt.so"
    try:
        lib = ctypes.CDLL(so_path)
    except OSError:
        return
    if not hasattr(lib, "axon_start_nrt_profile"):
        return
    lib.axon_start_nrt_profile.argtypes = [
        ctypes.POINTER(ctypes.c_int64), ctypes.c_size_t]
    lib.axon_start_nrt_profile.restype = ctypes.c_int64
    lib.axon_stop_nrt_profile.argtypes = [ctypes.c_char_p]
    lib.axon_stop_nrt_profile.restype = ctypes.c_int64

    @contextlib.contextmanager
    def _hook(output_dir, device_ids):
        import jax
        jax.devices()
        if device_ids:
            ids = (ctypes.c_int64 * len(device_ids))(*device_ids)
            rc = lib.axon_start_nrt_profile(ids, len(device_ids))
        else:
            rc = lib.axon_start_nrt_profile(None, 0)
        if rc != 0:
            raise RuntimeError(f"axon_start_nrt_profile rc={rc}")
        try:
            yield
        finally:
            n = lib.axon_stop_nrt_profile(str(output_dir).encode())
            print(f"profile: {n} file(s) written to {output_dir}")

    mod = types.ModuleType("antenv.axon_hooks")
    mod.get_axon_ntff_profile_hook = lambda: _hook
    mod.set_axon_ntff_profile_hook = lambda h: None
    sys.modules["antenv.axon_hooks"] = mod


_install_ntff_hook()


def _split_excess_waits(nc):
    """walrus CoreV3 codegen allows only one sync-wait on Drain instructions;
    Tile's tail drain aggregates one wait per outstanding semaphore.  Move the
    excess onto NOPs inserted just before the offender (same engine)."""
    for fn in nc.m.functions:
        for bb in fn.blocks:
            insts = bb.instructions
            i = 0
            while i < len(insts):
                inst = insts[i]
                si = inst.sync_info
                max_waits = 1
                if si is not None and si.on_wait and len(si.on_wait) > max_waits:
                    waits = list(si.on_wait)
                    si.on_wait = waits[:max_waits]
                    extra = waits[max_waits:]
                    new_nops = []
                    for j in range(0, len(extra), max_waits):
                        nop = nc.engines[inst.engine].nop(nofuse=True).ins
                        nop.sync_info = mybir.SyncInfo(
                            on_wait=extra[j : j + max_waits], on_update=[]
                        )
                        new_nops.append(nop)
                    for nop in new_nops:
                        for fb in fn.blocks:
                            if nop in fb.instructions:
                                fb.instructions.remove(nop)
                    idx = insts.index(inst)
                    for k, nop in enumerate(new_nops):
                        insts.insert(idx + k, nop)
                    i = insts.index(inst)
                i += 1


def _build_nc_v2(cc="a2a", pay="bf16"):
    nc = bass.Bass("TRN2", target_bir_lowering=False, debug=False,
                   num_devices=NCORES)
    PAY = BF16 if pay == "bf16" else F32

    xin = nc.dram_tensor("xin", [T, E], BF16, kind="ExternalInput")
    xa = nc.dram_tensor("xa", [T, ASH], F32, kind="ExternalInput")
    wt = nc.dram_tensor("wt", [S, S], BF16, kind="ExternalInput")
    cs = nc.dram_tensor("cs", [ASH, E, E], BF16, kind="ExternalInput")
    xres = nc.dram_tensor("xres", [128, E], F32, kind="ExternalInput")
    gw = nc.dram_tensor("gw", [128, E], F32, kind="ExternalInput")
    bw = nc.dram_tensor("bw", [128, E], F32, kind="ExternalInput")
    yout = nc.dram_tensor("yout", [128, E], F32, kind="ExternalOutput")

    TG = T // G
    ccin = [nc.dram_tensor(f"ccin{g}", [TG, E], PAY) for g in range(G)]
    if cc == "a2a":
        ccout = [nc.dram_tensor(f"a2a{g}", [TG, E], PAY) for g in range(G)]
    else:
        ccout = [nc.dram_tensor(f"rs{g}", [SPG, E], PAY) for g in range(G)]

    with tile.TileContext(nc) as tc:
        import contextlib
        with contextlib.ExitStack() as ctx:
            consts = ctx.enter_context(tc.tile_pool(name="consts", bufs=1))
            small = ctx.enter_context(tc.tile_pool(name="small", bufs=2))

            # ---------------- phase 0: async constant loads ------------------
            # two HWDGE rings (sync=SP, scalar=ACT); alternate the big C parts
            x_all = consts.tile([128, TCH, E], BF16, tag="x_all")
            nc.sync.dma_start(
                out=x_all, in_=xin.ap().rearrange("(tc p) c -> p tc c", p=128))
            wt_all = consts.tile([128, S // 128, S], BF16, tag="wt_all")
            nc.scalar.dma_start(
                out=wt_all, in_=wt.ap().rearrange("(j p) i -> p j i", p=128))
            xa_all = consts.tile([128, TCH, ASH], F32, tag="xa_all")
            nc.sync.dma_start(
                out=xa_all, in_=xa.ap().rearrange("(tc p) a -> p tc a", p=128))
            # resident concept-map slice: c_res[:, pc, a, :] = C[a, 128pc:, c]
            c_res = consts.tile([128, 2, ASH, E], BF16, tag="c_res")
            NCH = 4                       # 4 a-ranges x 2 pc = 8 x 512KB DMAs
            ACH = ASH // NCH
            for i in range(NCH):
                for pc in range(2):
                    src = cs.ap()[i * ACH:(i + 1) * ACH,
                                  pc * 128:(pc + 1) * 128, :]
                    eng = nc.sync if (2 * i + pc) % 2 == 0 else nc.scalar
                    eng.dma_start(
                        out=c_res[:, pc, i * ACH:(i + 1) * ACH, :],
                        in_=src.rearrange("a p c -> p a c"))
            xres_sb = consts.tile([RPG, G, E], F32, tag="xres")
            nc.scalar.dma_start(
                out=xres_sb, in_=xres.ap().rearrange("(g p) c -> p g c", p=RPG))
            gw_sb = consts.tile([RPG, E], F32, tag="gw")
            nc.scalar.dma_start(out=gw_sb, in_=gw.ap()[0:RPG, :])
            bw_sb = consts.tile([RPG, E], F32, tag="bw")
            nc.scalar.dma_start(out=bw_sb, in_=bw.ap()[0:RPG, :])

            # ---------------- phase 1: SmT = (W @ x)^T -----------------------
            # SmT[d, t] per batch: lhsT = x[b] j-chunk (j, d-slice), rhs = W^T
            smT = [consts.tile([128, T], BF16, tag=f"smT{d}", name=f"smT{d}")
                   for d in range(E // 128)]
            with tc.tile_pool(name="ps_s", bufs=2, space="PSUM") as ps_s:
                for b in range(B):
                    for d in range(E // 128):
                        ps = ps_s.tile([128, S], F32, tag="ps_s")
                        for j in range(S // 128):
                            nc.tensor.matmul(
                                ps,
                                lhsT=x_all[:, b * 4 + j, d * 128:(d + 1) * 128],
                                rhs=wt_all[:, j, :],
                                start=(j == 0),
                                stop=(j == S // 128 - 1),
                            )
                        nc.scalar.copy(smT[d][:, b * S:(b + 1) * S], ps)

            # ---------------- phase 2: grouped contraction -------------------
            ps_y = ctx.enter_context(
                tc.tile_pool(name="ps_y", bufs=4, space="PSUM"))
            y_acc = [consts.tile([128, E], F32, tag=f"yacc{t}",
                                 name=f"yacc{t}")
                     for t in range(TCH)]
            y_fin = consts.tile([128, TPG, E], PAY, tag="y_fin")

            for g in range(G):
                for ap_i in range(ASH // 2):          # a-pairs, C resident
                    for tl in range(TPG):
                        t = g * TPG + tl
                        ps = ps_y.tile([128, 2 * E], F32, tag="ps_y")
                        for pc in range(2):
                            nc.tensor.matmul(
                                ps,
                                lhsT=smT[pc][:, t * 128:(t + 1) * 128],
                                rhs=c_res[:, pc, 2 * ap_i:2 * ap_i + 2, :]
                                    .rearrange("p a c -> p (a c)"),
                                start=(pc == 0),
                                stop=(pc == 1),
                            )
                        for ai in range(2):
                            a = 2 * ap_i + ai
                            half = ps[:, ai * E:(ai + 1) * E]
                            xs = xa_all[:, t, a:a + 1]
                            if a == 0:
                                nc.vector.tensor_scalar(
                                    out=y_acc[t], in0=half, scalar1=xs,
                                    scalar2=None, op0=MULT)
                            elif a == ASH - 1:
                                nc.vector.scalar_tensor_tensor(
                                    out=y_fin[:, tl, :], in0=half, scalar=xs,
                                    in1=y_acc[t], op0=MULT, op1=ADD)
                            else:
                                nc.vector.scalar_tensor_tensor(
                                    out=y_acc[t], in0=half, scalar=xs,
                                    in1=y_acc[t], op0=MULT, op1=ADD)
                # ship group g partials, start collective
                nc.sync.dma_start(
                    out=ccin[g].ap().rearrange("(tc p) c -> p tc c", p=128),
                    in_=y_fin)
                nc.gpsimd.collective_compute(
                    "AllToAll" if cc == "a2a" else "ReduceScatter",
                    mybir.AluOpType.bypass if cc == "a2a" else ADD,
                    replica_groups=[list(range(NCORES))],
                    ins=[ccin[g].ap()],
                    outs=[ccout[g].ap()],
                )

            # ---------------- phase 3: per-group reduce + LN -----------------
            for g in range(G):
                if cc == "a2a":
                    gb = small.tile([RPG, NCORES, E], PAY, tag="gbuf")
                    nc.sync.dma_start(
                        out=gb,
                        in_=ccout[g].ap().rearrange("(s p) c -> p s c", p=RPG))
                    g4 = small.tile([RPG, 4, E], F32, tag="g4")
                    nc.vector.tensor_tensor(
                        out=g4, in0=gb[:, 0:4, :], in1=gb[:, 4:8, :], op=ADD)
                    g2 = small.tile([RPG, 2, E], F32, tag="g2")
                    nc.vector.tensor_tensor(
                        out=g2, in0=g4[:, 0:2, :], in1=g4[:, 2:4, :], op=ADD)
                    yred = small.tile([RPG, E], F32, tag="yred")
                    nc.vector.tensor_tensor(
                        out=yred, in0=g2[:, 0, :], in1=g2[:, 1, :], op=ADD)
                else:
                    yred = small.tile([RPG, E], F32, tag="yred")
                    nc.sync.dma_start(out=yred, in_=ccout[g].ap())
                r0 = g * RPG
                nc.vector.tensor_tensor(
                    out=yred, in0=yred, in1=xres_sb[:, g, :], op=ADD)
                stats = small.tile([RPG, 6], F32, tag="stats")
                nc.vector.bn_stats(out=stats, in_=yred)
                mv = small.tile([RPG, 2], F32, tag="mv")
                nc.vector.bn_aggr(out=mv, in_=stats)
                eps_t = small.tile([RPG, 1], F32, tag="eps")
                nc.vector.memset(eps_t, LN_EPS)
                std = small.tile([RPG, 1], F32, tag="std")
                nc.scalar.activation(out=std, in_=mv[:, 1:2], func=AF.Sqrt,
                                     bias=eps_t)
                rstd = small.tile([RPG, 1], F32, tag="rstd")
                nc.vector.reciprocal(out=rstd, in_=std)
                cent = small.tile([RPG, E], F32, tag="cent")
                nc.vector.tensor_scalar(
                    out=cent, in0=yred, scalar1=mv[:, 0:1], scalar2=None,
                    op0=mybir.AluOpType.subtract)
                tmp = small.tile([RPG, E], F32, tag="tmp")
                nc.vector.scalar_tensor_tensor(
                    out=tmp, in0=cent, scalar=rstd, in1=gw_sb,
                    op0=MULT, op1=MULT)
                yfin = small.tile([RPG, E], F32, tag="yfin")
                nc.vector.tensor_tensor(
                    out=yfin, in0=tmp, in1=bw_sb, op=ADD)
                nc.sync.dma_start(out=yout.ap()[r0:r0 + RPG, :], in_=yfin)

    _split_excess_waits(nc)
    return nc


def _build_nc_v3(cc="a2a", pay="bf16"):
    """Transposed phase 2: stationary = concept-map chunks, moving = x-scaled
    s^T; psum accumulates y^T over the whole (a, p) sweep per token group."""
    nc = bass.Bass("TRN2", target_bir_lowering=False, debug=False,
                   num_devices=NCORES)
    PAY = BF16 if pay == "bf16" else F32
    TG = T // G

    # all inputs in p-major layouts: per-partition data is one contiguous
    # run, so DMA descriptors are 4-32KB (small descriptors gut DMA rate)
    xin = nc.dram_tensor("xin", [128, TCH, E], BF16, kind="ExternalInput")
    xt = nc.dram_tensor("xt", [1, ASH * T], BF16, kind="ExternalInput")
    wt = nc.dram_tensor("wt", [128, S // 128, S], BF16, kind="ExternalInput")
    cs = nc.dram_tensor("cs", [2, 128, ASH, E], BF16, kind="ExternalInput")
    xres = nc.dram_tensor("xres", [RPG, G, E], F32, kind="ExternalInput")
    gw = nc.dram_tensor("gw", [RPG, E], F32, kind="ExternalInput")
    bw = nc.dram_tensor("bw", [RPG, E], F32, kind="ExternalInput")
    yout = nc.dram_tensor("yout", [128, E], F32, kind="ExternalOutput")

    ccin = [nc.dram_tensor(f"ccin{g}", [TG, E], PAY) for g in range(G)]
    if cc == "a2a":
        ccout = [nc.dram_tensor(f"a2a{g}", [TG, E], PAY) for g in range(G)]
    else:
        ccout = [nc.dram_tensor(f"rs{g}", [SPG, E], PAY) for g in range(G)]

    with tile.TileContext(nc) as tc:
        import contextlib
        with contextlib.ExitStack() as ctx:
            consts = ctx.enter_context(tc.tile_pool(name="consts", bufs=1))
            small = ctx.enter_context(tc.tile_pool(name="small", bufs=2))
            scp = ctx.enter_context(tc.tile_pool(name="scp", bufs=4))

            # ---------------- phase 0: async constant loads ------------------
            x_all = consts.tile([128, TCH, E], BF16, tag="x_all")
            nc.sync.dma_start(out=x_all, in_=xin.ap())
            wt_all = consts.tile([128, S // 128, S], BF16, tag="wt_all")
            nc.scalar.dma_start(out=wt_all, in_=wt.ap())
            xres_sb = consts.tile([RPG, G, E], F32, tag="xres")
            nc.scalar.dma_start(out=xres_sb, in_=xres.ap())
            gw_sb = consts.tile([RPG, E], F32, tag="gw")
            nc.scalar.dma_start(out=gw_sb, in_=gw.ap())
            bw_sb = consts.tile([RPG, E], F32, tag="bw")
            nc.scalar.dma_start(out=bw_sb, in_=bw.ap())

            # resident concept-map slice + host-replicated x-broadcast rows,
            # a-chunked DMAs issued in consumption order:
            #   c_res[:, pc, a, :] = C[a0+a, 128pc+p, c]
            #   bc_g[g][:, a, t]   = x[g*TG + t, a0+a]
            # x columns flattened into one partition: xt_flat[0, a*T + t]
            xt_flat = consts.tile([1, ASH * T], BF16, tag="xt_flat")
            nc.sync.dma_start(out=xt_flat, in_=xt.ap())
            ones_t = consts.tile([1, 128], BF16, tag="ones_t")
            nc.vector.memset(ones_t, 1.0)
            NCH = 4
            ACH = ASH // NCH
            c_ch = []
            for i in range(NCH):
                a0 = i * ACH
                ct = consts.tile([128, 2, ACH, E], BF16, tag=f"cch{i}",
                                 name=f"cch{i}")
                for pc in range(2):
                    eng = nc.sync if pc == 0 else nc.scalar
                    eng.dma_start(
                        out=ct[:, pc, :, :],
                        in_=cs.ap()[pc, :, a0:a0 + ACH, :])
                c_ch.append(ct)

            # ---------------- phase 1: SmT = (W @ x)^T -----------------------
            smT2 = consts.tile([128, 2, T], BF16, tag="smT2")
            with tc.tile_pool(name="ps_s", bufs=2, space="PSUM") as ps_s:
                for b in range(B):
                    for d in range(E // 128):
                        ps = ps_s.tile([128, S], F32, tag="ps_s")
                        tri = os.environ.get("KTRI", "1") == "1"
                        for j in range(S // 128):
                            # W^T[j, i] == 0 for i <= 128*j (lower-triangular)
                            i0 = 128 * j if (tri and j > 0) else 0
                            nc.tensor.matmul(
                                ps[:, i0:] if i0 else ps,
                                lhsT=x_all[:, b * 4 + j, d * 128:(d + 1) * 128],
                                rhs=wt_all[:, j, i0:] if i0 else wt_all[:, j, :],
                                start=(j == 0),
                                stop=(j == S // 128 - 1),
                            )
                        nc.scalar.copy(smT2[:, d, b * S:(b + 1) * S], ps)

            # ---------------- phase 2: transposed contraction ----------------
            ps_y = ctx.enter_context(
                tc.tile_pool(name="ps_y", bufs=2, space="PSUM"))
            y_fin = consts.tile([128, TPG, E], PAY, tag="y_fin")

            ps_bc = ctx.enter_context(
                tc.tile_pool(name="ps_bc", bufs=3, space="PSUM"))

            def bc_mm(g, a):
                # broadcast x[t0:t0+TG, a] to 128 partitions via a K=1 matmul
                bcps = ps_bc.tile([128, TG], F32, tag="bcps",
                                  name=f"bc{g}_{a}")
                nc.tensor.matmul(
                    bcps, lhsT=ones_t,
                    rhs=xt_flat[0:1, a * T + g * TG:a * T + (g + 1) * TG],
                    start=True, stop=True)
                return bcps

            for g in range(G):
                t0 = g * TG
                psum_yT = [ps_y.tile([128, TG], F32, tag=f"psyT{cc_}",
                                     name=f"psyT{g}_{cc_}")
                           for cc_ in range(2)]
                bcq = [bc_mm(g, 0), bc_mm(g, 1)]
                for a in range(ASH):
                    # one DVE op builds both p-chunks of the scaled s^T:
                    # sc2[:, pc, t] = smT2[:, pc, t0+t] * x[t0+t, a]
                    # (psum bc row broadcast across pc via a stride-0 dim)
                    sc2 = scp.tile([128, 2, TG], BF16, tag="sc2",
                                   name=f"sc{g}_{a}")
                    row = bcq.pop(0)
                    bcast = bass.AP(tensor=row.tensor, offset=row.offset,
                                    ap=[list(row.ap[0]), [0, 2],
                                        list(row.ap[1])])
                    nc.vector.tensor_tensor(
                        out=sc2, in0=smT2[:, :, t0:t0 + TG],
                        in1=bcast, op=MULT)
                    for cc_ in range(2):
                        for pc in range(2):
                            nc.tensor.matmul(
                                psum_yT[cc_],
                                lhsT=c_ch[a // ACH][:, pc, a % ACH,
                                                    cc_ * 128:(cc_ + 1) * 128],
                                rhs=sc2[:, pc, :],
                                start=(a == 0 and pc == 0),
                                stop=(a == ASH - 1 and pc == 1),
                            )
                    if a + 2 < ASH:
                        bcq.append(bc_mm(g, a + 2))
                # drain y^T (bf16) and xbar-transpose to a (p, tl)-row layout:
                # y_fin[p, tl, c] = y^T[c, t0 + p*TPG + tl]  (host un-permutes)
                yT = []
                for cc_ in range(2):
                    y_t = small.tile([128, TG], BF16, tag=f"yT{cc_}",
                                     name=f"yT{g}_{cc_}")
                    nc.scalar.copy(y_t, psum_yT[cc_])
                    yT.append(y_t)
                for cc_ in range(2):
                    eng = nc.sync if cc_ == 0 else nc.scalar
                    eng.dma_start_transpose(
                        out=y_fin[:, :, cc_ * 128:(cc_ + 1) * 128],
                        in_=yT[cc_])
                nc.sync.dma_start(
                    out=ccin[g].ap().rearrange("(tc p) c -> p tc c", p=128),
                    in_=y_fin)
                nc.gpsimd.collective_compute(
                    "AllToAll" if cc == "a2a" else "ReduceScatter",
                    mybir.AluOpType.bypass if cc == "a2a" else ADD,
                    replica_groups=[list(range(NCORES))],
                    ins=[ccin[g].ap()],
                    outs=[ccout[g].ap()],
                )

            # ---------------- phase 3: per-group reduce + LN -----------------
            for g in range(G):
                if cc == "a2a":
                    gb = small.tile([RPG, NCORES, E], PAY, tag="gbuf")
                    nc.gpsimd.dma_start(
                        out=gb,
                        in_=ccout[g].ap().rearrange("(s p) c -> p s c", p=RPG))
                    g4 = small.tile([RPG, 4, E], F32, tag="g4")
                    nc.vector.tensor_tensor(
                        out=g4, in0=gb[:, 0:4, :], in1=gb[:, 4:8, :], op=ADD)
                    g2 = small.tile([RPG, 2, E], F32, tag="g2")
                    nc.vector.tensor_tensor(
                        out=g2, in0=g4[:, 0:2, :], in1=g4[:, 2:4, :], op=ADD)
                    yred = small.tile([RPG, E], F32, tag="yred")
                    nc.vector.tensor_tensor(
                        out=yred, in0=g2[:, 0, :], in1=g2[:, 1, :], op=ADD)
                else:
                    yred = small.tile([RPG, E], F32, tag="yred")
                    nc.gpsimd.dma_start(out=yred, in_=ccout[g].ap())
                r0 = g * RPG
                nc.vector.tensor_tensor(
                    out=yred, in0=yred, in1=xres_sb[:, g, :], op=ADD)
                stats = small.tile([RPG, 6], F32, tag="stats")
                nc.vector.bn_stats(out=stats, in_=yred)
                mv = small.tile([RPG, 2], F32, tag="mv")
                nc.vector.bn_aggr(out=mv, in_=stats)
                eps_t = small.tile([RPG, 1], F32, tag="eps")
                nc.vector.memset(eps_t, LN_EPS)
                std = small.tile([RPG, 1], F32, tag="std")
                nc.scalar.activation(out=std, in_=mv[:, 1:2], func=AF.Sqrt,
                                     bias=eps_t)
                rstd = small.tile([RPG, 1], F32, tag="rstd")
                nc.vector.reciprocal(out=rstd, in_=std)
                cent = small.tile([RPG, E], F32, tag="cent")
                nc.vector.tensor_scalar(
                    out=cent, in0=yred, scalar1=mv[:, 0:1], scalar2=None,
                    op0=mybir.AluOpType.subtract)
                tmp = small.tile([RPG, E], F32, tag="tmp")
                nc.vector.scalar_tensor_tensor(
                    out=tmp, in0=cent, scalar=rstd, in1=gw_sb,
                    op0=MULT, op1=MULT)
                yfin = small.tile([RPG, E], F32, tag="yfin")
                nc.vector.tensor_tensor(
                    out=yfin, in0=tmp, in1=bw_sb, op=ADD)
                nc.scalar.dma_start(out=yout.ap()[r0:r0 + RPG, :], in_=yfin)

    _split_excess_waits(nc)
    return nc


def _get_nc(mode, cc, pay):
    key = (mode, cc, pay, G)
    if key not in _NC_CACHE:
        if mode == "v2":
            _NC_CACHE[key] = _build_nc_v2(cc, pay)
        elif mode == "v3":
            _NC_CACHE[key] = _build_nc_v3(cc, pay)
        else:
            raise ValueError(f"unknown mode {mode}")
    return _NC_CACHE[key]


def _prefix_wt():
    idx = np.arange(S)
    diff = idx[:, None] - idx[None, :]          # i - j
    W = np.where(diff > 0, 1.0 / np.square(np.maximum(diff, 1)), 0.0)
    return np.ascontiguousarray(W.T.astype(np.float32))   # WT[j, i] = W[i, j]


def kernel(x, concept_map, gamma, beta, mode=None, trace=False):
    global LAST_RESULTS
    mode = mode or MODE
    import ml_dtypes  # noqa: F401  (registers bfloat16 with numpy)
    xf = np.ascontiguousarray(np.asarray(x, dtype=np.float32).reshape(T, E))
    cmap = np.asarray(concept_map, dtype=np.float32)
    gammaf = np.asarray(gamma, dtype=np.float32)
    betaf = np.asarray(beta, dtype=np.float32)

    wt_np = _prefix_wt().astype(ml_dtypes.bfloat16)
    # C_perm[a, p, c] = concept_map[c, a, p]
    cperm = np.ascontiguousarray(np.transpose(cmap, (1, 2, 0)))
    x_bf = xf.astype(ml_dtypes.bfloat16)

    TG = T // G
    # p-major packings shared across cores
    # xin[p, tc, c] = x[tc*128+p, c];  wt[p, j, i] = W^T[j*128+p, i]
    xin_p = np.ascontiguousarray(
        x_bf.reshape(TCH, 128, E).transpose(1, 0, 2))
    wt_p = np.ascontiguousarray(
        wt_np.reshape(S // 128, 128, S).transpose(1, 0, 2))
    gb = np.ascontiguousarray(np.broadcast_to(gammaf, (RPG, E))).astype(
        np.float32)
    bb = np.ascontiguousarray(np.broadcast_to(betaf, (RPG, E))).astype(
        np.float32)

    # ccin row r of group g holds token g*TG + (r%128)*TPG + r//128 (the
    # on-device transpose writes (p, tl)-major rows); core c owns rows
    # [SPG*c, SPG*(c+1)) of each group
    TPGh = TCH // G
    perm = os.environ.get("KPERM", "tp")
    own_tok = np.empty((NCORES, G, SPG), dtype=np.int64)
    for c in range(NCORES):
        for g in range(G):
            r = SPG * c + np.arange(SPG)
            if perm == "pt":
                own_tok[c, g] = g * TG + (r % 128) * TPGh + r // 128
            else:
                own_tok[c, g] = g * TG + r

    in_maps = []
    for c in range(NCORES):
        a0 = c * ASH
        own = np.stack([xf[own_tok[c, g]] for g in range(G)],
                       axis=1)  # [RPG, G, E]
        im = {
            "xin": xin_p,
            "wt": wt_p,
            "xres": np.ascontiguousarray(own),
            "gw": gb,
            "bw": bb,
        }
        if mode == "v2":
            im["cs"] = np.ascontiguousarray(cperm[a0:a0 + ASH]).astype(
                ml_dtypes.bfloat16)
            im["xa"] = np.ascontiguousarray(xf[:, a0:a0 + ASH])
        else:
            # cs[pc, p, a, c] = C_perm[a0+a, pc*128+p, c]
            cslice = cperm[a0:a0 + ASH].astype(ml_dtypes.bfloat16)
            im["cs"] = np.ascontiguousarray(
                cslice.reshape(ASH, 2, 128, E).transpose(1, 2, 0, 3))
            # xt[0, a*T + t] = x[t, a0 + a]
            im["xt"] = np.ascontiguousarray(
                x_bf[:, a0:a0 + ASH].T.reshape(1, ASH * T))
        in_maps.append(im)

    cc = os.environ.get("KCC", "a2a")
    pay = os.environ.get("KPAY", "bf16")
    nc = _get_nc(mode, cc, pay)
    res = None
    for attempt in range(4):
        try:
            res = run_bass_kernel_spmd(nc, in_maps, list(range(NCORES)),
                                       trace=trace)
            break
        except Exception:
            # transient NRT_EXEC_UNIT_UNRECOVERABLE happens occasionally on
            # the first dispatch after a fresh compile; back off and retry
            if attempt == 3:
                raise
            import time
            time.sleep(10 * (attempt + 1))
    LAST_RESULTS = res
    out = np.empty((T, E), dtype=np.float32)
    for c in range(NCORES):
        yc = res.results[c]["yout"]
        for g in range(G):
            out[own_tok[c, g]] = yc[g * RPG:(g + 1) * RPG]
    return np.ascontiguousarray(out.reshape(B, S, E).astype(np.float32))


# revision 52
# speedup vs baseline: 1.7311x; 1.7311x over previous
"""Trainium2 Bass kernel for nn_ConceptLayer (B=2, S=512, E=256), 8 NeuronCores.

Math:
  s[b,i,:] = sum_{j<i} x[b,j,:] / (i-j)^2            (prefix matmul, W constant)
  y[b,i,c] = sum_{a,p} x[b,i,a] * s[b,i,p] * C[c,a,p]
  out      = LayerNorm(y + x) * gamma + beta          (eps=1e-3)

Sharding: contraction axis `a` split 8 ways (32 a's per core); each core streams
1/8 of the concept_map (bf16, 4MB) once into SBUF and keeps it resident.

v2 device algorithm per core:
  phase 1: SmT[p,t] = s^T via PE (x bf16 stationary, W^T bf16 moving) -> bf16
  phase 2: token-group pipeline (G groups of T/G tokens):
      for each a-pair: psum[t,2*E] = SmT^T @ [C_a0|C_a1]  (bf16 matmul, N=512)
      DVE: y[t] (+)= x[t,a] * psum-half   (scalar_tensor_tensor, per-partition
           scalar; final a writes bf16 y_fin directly)
      after each group: DMA y_fin -> ccin_g, AllToAll (bf16, overlapped with the
      next group's matmuls), local 8-way reduce, +x residual, LayerNorm, store.
Host gathers the 8 [128,E] outputs and reorders token slices.
"""
import os
import numpy as np

import concourse.bass as bass
import concourse.mybir as mybir
import concourse.tile as tile
from concourse.bass_utils import run_bass_kernel_spmd

# ----------------------------------------------------------------------------
# constants (hardcoded per problem spec)
B, S, E = 2, 512, 256
T = B * S                      # 1024 tokens
NCORES = 8
ASH = E // NCORES              # 32 contraction-a values per core
TCH = T // 128                 # 8 token chunks of 128
LN_EPS = 1e-3

G = int(os.environ.get("KG", "2"))          # token groups (collective staging)
TPG = TCH // G                 # t-chunks per group
RPG = 128 // G                 # output rows per (core, group)
SPG = T // G // NCORES         # tokens per (core, group) == RPG

F32 = mybir.dt.float32
BF16 = mybir.dt.bfloat16
MULT = mybir.AluOpType.mult
ADD = mybir.AluOpType.add
AF = mybir.ActivationFunctionType

MODE = os.environ.get("KMODE", "v3")

LAST_RESULTS = None            # BassKernelResults of the last run (for test.py)

_NC_CACHE = {}


def _install_ntff_hook():
    """antenv.axon_hooks is absent in this image; recreate it so
    run_bass_kernel_spmd(trace=True) can drive NTFF profiling via the
    libaxon_pjrt.so C ABI (same recipe as trn_agent_boot)."""
    import sys, types, ctypes, contextlib  # noqa: E401

    if "antenv.axon_hooks" in sys.modules:
        return
    so_path = "/opt/axon/libaxon_pjrt.so"
    try:
        lib = ctypes.CDLL(so_path)
    except OSError:
        return
    if not hasattr(lib, "axon_start_nrt_profile"):
        return
    lib.axon_start_nrt_profile.argtypes = [
        ctypes.POINTER(ctypes.c_int64), ctypes.c_size_t]
    lib.axon_start_nrt_profile.restype = ctypes.c_int64
    lib.axon_stop_nrt_profile.argtypes = [ctypes.c_char_p]
    lib.axon_stop_nrt_profile.restype = ctypes.c_int64

    @contextlib.contextmanager
    def _hook(output_dir, device_ids):
        import jax
        jax.devices()
        if device_ids:
            ids = (ctypes.c_int64 * len(device_ids))(*device_ids)
            rc = lib.axon_start_nrt_profile(ids, len(device_ids))
        else:
            rc = lib.axon_start_nrt_profile(None, 0)
        if rc != 0:
            raise RuntimeError(f"axon_start_nrt_profile rc={rc}")
        try:
            yield
        finally:
            n = lib.axon_stop_nrt_profile(str(output_dir).encode())
            print(f"profile: {n} file(s) written to {output_dir}")

    mod = types.ModuleType("antenv.axon_hooks")
    mod.get_axon_ntff_profile_hook = lambda: _hook
    mod.set_axon_ntff_profile_hook = lambda h: None
    sys.modules["antenv.axon_hooks"] = mod


_install_ntff_hook()


def _split_excess_waits(nc):
    """walrus CoreV3 codegen allows only one sync-wait on Drain instructions;
    Tile's tail drain aggregates one wait per outstanding semaphore.  Move the
    excess onto NOPs inserted just before the offender (same engine)."""
    for fn in nc.m.functions:
        for bb in fn.blocks:
            insts = bb.instructions
            i = 0
            while i < len(insts):
                inst = insts[i]
                si = inst.sync_info
                max_waits = 1
                if si is not None and si.on_wait and len(si.on_wait) > max_waits:
                    waits = list(si.on_wait)
                    si.on_wait = waits[:max_waits]
                    extra = waits[max_waits:]
                    new_nops = []
                    for j in range(0, len(extra), max_waits):
                        nop = nc.engines[inst.engine].nop(nofuse=True).ins
                        nop.sync_info = mybir.SyncInfo(
                            on_wait=extra[j : j + max_waits], on_update=[]
                        )
                        new_nops.append(nop)
                    for nop in new_nops:
                        for fb in fn.blocks:
                            if nop in fb.instructions:
                                fb.instructions.remove(nop)
                    idx = insts.index(inst)
                    for k, nop in enumerate(new_nops):
                        insts.insert(idx + k, nop)
                    i = insts.index(inst)
                i += 1


def _build_nc_v2(cc="a2a", pay="bf16"):
    nc = bass.Bass("TRN2", target_bir_lowering=False, debug=False,
                   num_devices=NCORES)
    PAY = BF16 if pay == "bf16" else F32

    xin = nc.dram_tensor("xin", [T, E], BF16, kind="ExternalInput")
    xa = nc.dram_tensor("xa", [T, ASH], F32, kind="ExternalInput")
    wt = nc.dram_tensor("wt", [S, S], BF16, kind="ExternalInput")
    cs = nc.dram_tensor("cs", [ASH, E, E], BF16, kind="ExternalInput")
    xres = nc.dram_tensor("xres", [128, E], F32, kind="ExternalInput")
    gw = nc.dram_tensor("gw", [128, E], F32, kind="ExternalInput")
    bw = nc.dram_tensor("bw", [128, E], F32, kind="ExternalInput")
    yout = nc.dram_tensor("yout", [128, E], F32, kind="ExternalOutput")

    TG = T // G
    ccin = [nc.dram_tensor(f"ccin{g}", [TG, E], PAY) for g in range(G)]
    if cc == "a2a":
        ccout = [nc.dram_tensor(f"a2a{g}", [TG, E], PAY) for g in range(G)]
    else:
        ccout = [nc.dram_tensor(f"rs{g}", [SPG, E], PAY) for g in range(G)]

    with tile.TileContext(nc) as tc:
        import contextlib
        with contextlib.ExitStack() as ctx:
            consts = ctx.enter_context(tc.tile_pool(name="consts", bufs=1))
            small = ctx.enter_context(tc.tile_pool(name="small", bufs=2))

            # ---------------- phase 0: async constant loads ------------------
            # two HWDGE rings (sync=SP, scalar=ACT); alternate the big C parts
            x_all = consts.tile([128, TCH, E], BF16, tag="x_all")
            nc.sync.dma_start(
                out=x_all, in_=xin.ap().rearrange("(tc p) c -> p tc c", p=128))
            wt_all = consts.tile([128, S // 128, S], BF16, tag="wt_all")
            nc.scalar.dma_start(
                out=wt_all, in_=wt.ap().rearrange("(j p) i -> p j i", p=128))
            xa_all = consts.tile([128, TCH, ASH], F32, tag="xa_all")
            nc.sync.dma_start(
                out=xa_all, in_=xa.ap().rearrange("(tc p) a -> p tc a", p=128))
            # resident concept-map slice: c_res[:, pc, a, :] = C[a, 128pc:, c]
            c_res = consts.tile([128, 2, ASH, E], BF16, tag="c_res")
            NCH = 4                       # 4 a-ranges x 2 pc = 8 x 512KB DMAs
            ACH = ASH // NCH
            for i in range(NCH):
                for pc in range(2):
                    src = cs.ap()[i * ACH:(i + 1) * ACH,
                                  pc * 128:(pc + 1) * 128, :]
                    eng = nc.sync if (2 * i + pc) % 2 == 0 else nc.scalar
                    eng.dma_start(
                        out=c_res[:, pc, i * ACH:(i + 1) * ACH, :],
                        in_=src.rearrange("a p c -> p a c"))
            xres_sb = consts.tile([RPG, G, E], F32, tag="xres")
            nc.scalar.dma_start(
                out=xres_sb, in_=xres.ap().rearrange("(g p) c -> p g c", p=RPG))
            gw_sb = consts.tile([RPG, E], F32, tag="gw")
            nc.scalar.dma_start(out=gw_sb, in_=gw.ap()[0:RPG, :])
            bw_sb = consts.tile([RPG, E], F32, tag="bw")
            nc.scalar.dma_start(out=bw_sb, in_=bw.ap()[0:RPG, :])

            # ---------------- phase 1: SmT = (W @ x)^T -----------------------
            # SmT[d, t] per batch: lhsT = x[b] j-chunk (j, d-slice), rhs = W^T
            smT = [consts.tile([128, T], BF16, tag=f"smT{d}", name=f"smT{d}")
                   for d in range(E // 128)]
            with tc.tile_pool(name="ps_s", bufs=2, space="PSUM") as ps_s:
                for b in range(B):
                    for d in range(E // 128):
                        ps = ps_s.tile([128, S], F32, tag="ps_s")
                        for j in range(S // 128):
                            nc.tensor.matmul(
                                ps,
                                lhsT=x_all[:, b * 4 + j, d * 128:(d + 1) * 128],
                                rhs=wt_all[:, j, :],
                                start=(j == 0),
                                stop=(j == S // 128 - 1),
                            )
                        nc.scalar.copy(smT[d][:, b * S:(b + 1) * S], ps)

            # ---------------- phase 2: grouped contraction -------------------
            ps_y = ctx.enter_context(
                tc.tile_pool(name="ps_y", bufs=4, space="PSUM"))
            y_acc = [consts.tile([128, E], F32, tag=f"yacc{t}",
                                 name=f"yacc{t}")
                     for t in range(TCH)]
            y_fin = consts.tile([128, TPG, E], PAY, tag="y_fin")

            for g in range(G):
                for ap_i in range(ASH // 2):          # a-pairs, C resident
                    for tl in range(TPG):
                        t = g * TPG + tl
                        ps = ps_y.tile([128, 2 * E], F32, tag="ps_y")
                        for pc in range(2):
                            nc.tensor.matmul(
                                ps,
                                lhsT=smT[pc][:, t * 128:(t + 1) * 128],
                                rhs=c_res[:, pc, 2 * ap_i:2 * ap_i + 2, :]
                                    .rearrange("p a c -> p (a c)"),
                                start=(pc == 0),
                                stop=(pc == 1),
                            )
                        for ai in range(2):
                            a = 2 * ap_i + ai
                            half = ps[:, ai * E:(ai + 1) * E]
                            xs = xa_all[:, t, a:a + 1]
                            if a == 0:
                                nc.vector.tensor_scalar(
                                    out=y_acc[t], in0=half, scalar1=xs,
                                    scalar2=None, op0=MULT)
                            elif a == ASH - 1:
                                nc.vector.scalar_tensor_tensor(
                                    out=y_fin[:, tl, :], in0=half, scalar=xs,
                                    in1=y_acc[t], op0=MULT, op1=ADD)
                            else:
                                nc.vector.scalar_tensor_tensor(
                                    out=y_acc[t], in0=half, scalar=xs,
                                    in1=y_acc[t], op0=MULT, op1=ADD)
                # ship group g partials, start collective
                nc.sync.dma_start(
                    out=ccin[g].ap().rearrange("(tc p) c -> p tc c", p=128),
                    in_=y_fin)
                nc.gpsimd.collective_compute(
                    "AllToAll" if cc == "a2a" else "ReduceScatter",
                    mybir.AluOpType.bypass if cc == "a2a" else ADD,
                    replica_groups=[list(range(NCORES))],
                    ins=[ccin[g].ap()],
                    outs=[ccout[g].ap()],
                )

            # ---------------- phase 3: per-group reduce + LN -----------------
            for g in range(G):
                if cc == "a2a":
                    gb = small.tile([RPG, NCORES, E], PAY, tag="gbuf")
                    nc.sync.dma_start(
                        out=gb,
                        in_=ccout[g].ap().rearrange("(s p) c -> p s c", p=RPG))
                    g4 = small.tile([RPG, 4, E], F32, tag="g4")
                    nc.vector.tensor_tensor(
                        out=g4, in0=gb[:, 0:4, :], in1=gb[:, 4:8, :], op=ADD)
                    g2 = small.tile([RPG, 2, E], F32, tag="g2")
                    nc.vector.tensor_tensor(
                        out=g2, in0=g4[:, 0:2, :], in1=g4[:, 2:4, :], op=ADD)
                    yred = small.tile([RPG, E], F32, tag="yred")
                    nc.vector.tensor_tensor(
                        out=yred, in0=g2[:, 0, :], in1=g2[:, 1, :], op=ADD)
                else:
                    yred = small.tile([RPG, E], F32, tag="yred")
                    nc.sync.dma_start(out=yred, in_=ccout[g].ap())
                r0 = g * RPG
                nc.vector.tensor_tensor(
                    out=yred, in0=yred, in1=xres_sb[:, g, :], op=ADD)
                stats = small.tile([RPG, 6], F32, tag="stats")
                nc.vector.bn_stats(out=stats, in_=yred)
                mv = small.tile([RPG, 2], F32, tag="mv")
                nc.vector.bn_aggr(out=mv, in_=stats)
                eps_t = small.tile([RPG, 1], F32, tag="eps")
                nc.vector.memset(eps_t, LN_EPS)
                std = small.tile([RPG, 1], F32, tag="std")
                nc.scalar.activation(out=std, in_=mv[:, 1:2], func=AF.Sqrt,
                                     bias=eps_t)
                rstd = small.tile([RPG, 1], F32, tag="rstd")
                nc.vector.reciprocal(out=rstd, in_=std)
                cent = small.tile([RPG, E], F32, tag="cent")
                nc.vector.tensor_scalar(
                    out=cent, in0=yred, scalar1=mv[:, 0:1], scalar2=None,
                    op0=mybir.AluOpType.subtract)
                tmp = small.tile([RPG, E], F32, tag="tmp")
                nc.vector.scalar_tensor_tensor(
                    out=tmp, in0=cent, scalar=rstd, in1=gw_sb,
                    op0=MULT, op1=MULT)
                yfin = small.tile([RPG, E], F32, tag="yfin")
                nc.vector.tensor_tensor(
                    out=yfin, in0=tmp, in1=bw_sb, op=ADD)
                nc.sync.dma_start(out=yout.ap()[r0:r0 + RPG, :], in_=yfin)

    _split_excess_waits(nc)
    return nc


def _build_nc_v3(cc="a2a", pay="bf16"):
    """Transposed phase 2: stationary = concept-map chunks, moving = x-scaled
    s^T; psum accumulates y^T over the whole (a, p) sweep per token group."""
    nc = bass.Bass("TRN2", target_bir_lowering=False, debug=False,
                   num_devices=NCORES)
    PAY = BF16 if pay == "bf16" else F32
    TG = T // G

    # all inputs in p-major layouts: per-partition data is one contiguous
    # run, so DMA descriptors are 4-32KB (small descriptors gut DMA rate)
    xin = nc.dram_tensor("xin", [128, TCH, E], BF16, kind="ExternalInput")
    xbc = nc.dram_tensor("xbc", [G, 128, ASH, T // G], BF16,
                         kind="ExternalInput")
    wt = nc.dram_tensor("wt", [128, S // 128, S], BF16, kind="ExternalInput")
    cs = nc.dram_tensor("cs", [2, 128, ASH, E], BF16, kind="ExternalInput")
    xres = nc.dram_tensor("xres", [RPG, G, E], F32, kind="ExternalInput")
    gw = nc.dram_tensor("gw", [RPG, E], F32, kind="ExternalInput")
    bw = nc.dram_tensor("bw", [RPG, E], F32, kind="ExternalInput")
    ident = nc.dram_tensor("ident", [128, 128], BF16, kind="ExternalInput")
    yout = nc.dram_tensor("yout", [128, E], F32, kind="ExternalOutput")

    ccin = [nc.dram_tensor(f"ccin{g}", [TG, E], PAY) for g in range(G)]
    if cc == "a2a":
        ccout = [nc.dram_tensor(f"a2a{g}", [TG, E], PAY) for g in range(G)]
    else:
        ccout = [nc.dram_tensor(f"rs{g}", [SPG, E], PAY) for g in range(G)]

    with tile.TileContext(nc) as tc:
        import contextlib
        with contextlib.ExitStack() as ctx:
            consts = ctx.enter_context(tc.tile_pool(name="consts", bufs=1))
            small = ctx.enter_context(tc.tile_pool(name="small", bufs=2))
            scp = ctx.enter_context(tc.tile_pool(name="scp", bufs=4))

            # ---------------- phase 0: async constant loads ------------------
            x_all = consts.tile([128, TCH, E], BF16, tag="x_all")
            nc.sync.dma_start(out=x_all, in_=xin.ap())
            wt_all = consts.tile([128, S // 128, S], BF16, tag="wt_all")
            nc.scalar.dma_start(out=wt_all, in_=wt.ap())
            xres_sb = consts.tile([RPG, G, E], F32, tag="xres")
            nc.scalar.dma_start(out=xres_sb, in_=xres.ap())
            gw_sb = consts.tile([RPG, E], F32, tag="gw")
            nc.scalar.dma_start(out=gw_sb, in_=gw.ap())
            bw_sb = consts.tile([RPG, E], F32, tag="bw")
            nc.scalar.dma_start(out=bw_sb, in_=bw.ap())
            idn = consts.tile([128, 128], BF16, tag="idn")
            nc.scalar.dma_start(out=idn, in_=ident.ap())

            # resident concept-map slice + host-replicated x-broadcast rows,
            # a-chunked DMAs issued in consumption order:
            #   c_res[:, pc, a, :] = C[a0+a, 128pc+p, c]
            #   bc_g[g][:, a, t]   = x[g*TG + t, a0+a]
            NCH = 4
            ACH = ASH // NCH
            c_ch = []
            bc_ch = []
            for i in range(NCH):
                a0 = i * ACH
                ct = consts.tile([128, 2, ACH, E], BF16, tag=f"cch{i}",
                                 name=f"cch{i}")
                for pc in range(2):
                    eng = nc.sync if pc == 0 else nc.scalar
                    eng.dma_start(
                        out=ct[:, pc, :, :],
                        in_=cs.ap()[pc, :, a0:a0 + ACH, :])
                c_ch.append(ct)
                bt = consts.tile([128, ACH, TG], BF16, tag=f"bc{i}_0",
                                 name=f"bc{i}_0")
                eng = nc.sync if i % 2 == 0 else nc.scalar
                eng.dma_start(out=bt, in_=xbc.ap()[0, :, a0:a0 + ACH, :])
                bc_ch.append([bt])
            # group >0 broadcast rows can land during the group-0 sweep
            for g in range(1, G):
                for i in range(NCH):
                    a0 = i * ACH
                    bt = consts.tile([128, ACH, TG], BF16, tag=f"bc{i}_{g}",
                                     name=f"bc{i}_{g}")
                    eng = nc.sync if i % 2 == 0 else nc.scalar
                    eng.dma_start(out=bt, in_=xbc.ap()[g, :, a0:a0 + ACH, :])
                    bc_ch[i].append(bt)

            # ---------------- phase 1: SmT = (W @ x)^T -----------------------
            smT2 = consts.tile([128, 2, T], BF16, tag="smT2")
            with tc.tile_pool(name="ps_s", bufs=2, space="PSUM") as ps_s:
                for b in range(B):
                    for d in range(E // 128):
                        ps = ps_s.tile([128, S], F32, tag="ps_s")
                        tri = os.environ.get("KTRI", "1") == "1"
                        for j in range(S // 128):
                            # W^T[j, i] == 0 for i <= 128*j (lower-triangular)
                            i0 = 128 * j if (tri and j > 0) else 0
                            nc.tensor.matmul(
                                ps[:, i0:] if i0 else ps,
                                lhsT=x_all[:, b * 4 + j, d * 128:(d + 1) * 128],
                                rhs=wt_all[:, j, i0:] if i0 else wt_all[:, j, :],
                                start=(j == 0),
                                stop=(j == S // 128 - 1),
                            )
                        nc.scalar.copy(smT2[:, d, b * S:(b + 1) * S], ps)

            # ---------------- phase 2: transposed contraction ----------------
            ps_y = ctx.enter_context(
                tc.tile_pool(name="ps_y", bufs=2, space="PSUM"))
            y_fin = consts.tile([128, TPG, E], PAY, tag="y_fin")

            for g in range(G):
                t0 = g * TG
                psum_yT = [ps_y.tile([128, TG], F32, tag=f"psyT{cc_}",
                                     name=f"psyT{g}_{cc_}")
                           for cc_ in range(2)]
                for a in range(ASH):
                    # one DVE op builds both p-chunks of the scaled s^T:
                    # sc2[:, pc, t] = smT2[:, pc, t0+t] * x[t0+t, a]
                    # (bc row broadcast across pc via a stride-0 middle dim)
                    sc2 = scp.tile([128, 2, TG], BF16, tag="sc2",
                                   name=f"sc{g}_{a}")
                    row = bc_ch[a // ACH][g][:, a % ACH, :]
                    bcast = bass.AP(tensor=row.tensor, offset=row.offset,
                                    ap=[list(row.ap[0]), [0, 2],
                                        list(row.ap[1])])
                    nc.vector.tensor_tensor(
                        out=sc2, in0=smT2[:, :, t0:t0 + TG],
                        in1=bcast, op=MULT)
                    for cc_ in range(2):
                        for pc in range(2):
                            nc.tensor.matmul(
                                psum_yT[cc_],
                                lhsT=c_ch[a // ACH][:, pc, a % ACH,
                                                    cc_ * 128:(cc_ + 1) * 128],
                                rhs=sc2[:, pc, :],
                                start=(a == 0 and pc == 0),
                                stop=(a == ASH - 1 and pc == 1),
                            )
                # drain y^T (bf16), transpose 128x128 blocks on the PE:
                # y_fin[p, tl, c] = y^T[c, t0 + tl*128 + p]
                yT = []
                for cc_ in range(2):
                    y_t = small.tile([128, TG], BF16, tag=f"yT{cc_}",
                                     name=f"yT{g}_{cc_}")
                    nc.scalar.copy(y_t, psum_yT[cc_])
                    yT.append(y_t)
                with tc.tile_pool(name=f"pxp{g}", bufs=2,
                                  space="PSUM") as pxp:
                    for tl in range(TPG):
                        for cc_ in range(2):
                            pt = pxp.tile([128, 128], BF16, tag="pxp")
                            nc.tensor.transpose(
                                pt, in_=yT[cc_][:, tl * 128:(tl + 1) * 128],
                                identity=idn)
                            nc.scalar.copy(
                                y_fin[:, tl, cc_ * 128:(cc_ + 1) * 128], pt)
                # ccin linear row l = p*TPG + tc holds token t0 + tc*128 + p
                nc.sync.dma_start(
                    out=ccin[g].ap().rearrange("(p tc) c -> p tc c", p=128),
                    in_=y_fin)
                nc.gpsimd.collective_compute(
                    "AllToAll" if cc == "a2a" else "ReduceScatter",
                    mybir.AluOpType.bypass if cc == "a2a" else ADD,
                    replica_groups=[list(range(NCORES))],
                    ins=[ccin[g].ap()],
                    outs=[ccout[g].ap()],
                )

            # ---------------- phase 3: per-group reduce + LN -----------------
            for g in range(G):
                if cc == "a2a":
                    gb = small.tile([RPG, NCORES, E], PAY, tag="gbuf")
                    nc.gpsimd.dma_start(
                        out=gb,
                        in_=ccout[g].ap().rearrange("(s p) c -> p s c", p=RPG))
                    g4 = small.tile([RPG, 4, E], F32, tag="g4")
                    nc.vector.tensor_tensor(
                        out=g4, in0=gb[:, 0:4, :], in1=gb[:, 4:8, :], op=ADD)
                    g2 = small.tile([RPG, 2, E], F32, tag="g2")
                    nc.vector.tensor_tensor(
                        out=g2, in0=g4[:, 0:2, :], in1=g4[:, 2:4, :], op=ADD)
                    yred = small.tile([RPG, E], F32, tag="yred")
                    nc.vector.tensor_tensor(
                        out=yred, in0=g2[:, 0, :], in1=g2[:, 1, :], op=ADD)
                else:
                    yred = small.tile([RPG, E], F32, tag="yred")
                    nc.gpsimd.dma_start(out=yred, in_=ccout[g].ap())
                r0 = g * RPG
                nc.vector.tensor_tensor(
                    out=yred, in0=yred, in1=xres_sb[:, g, :], op=ADD)
                stats = small.tile([RPG, 6], F32, tag="stats")
                nc.vector.bn_stats(out=stats, in_=yred)
                mv = small.tile([RPG, 2], F32, tag="mv")
                nc.vector.bn_aggr(out=mv, in_=stats)
                eps_t = small.tile([RPG, 1], F32, tag="eps")
                nc.vector.memset(eps_t, LN_EPS)
                std = small.tile([RPG, 1], F32, tag="std")
                nc.scalar.activation(out=std, in_=mv[:, 1:2], func=AF.Sqrt,
                                     bias=eps_t)
                rstd = small.tile([RPG, 1], F32, tag="rstd")
                nc.vector.reciprocal(out=rstd, in_=std)
                cent = small.tile([RPG, E], F32, tag="cent")
                nc.vector.tensor_scalar(
                    out=cent, in0=yred, scalar1=mv[:, 0:1], scalar2=None,
                    op0=mybir.AluOpType.subtract)
                tmp = small.tile([RPG, E], F32, tag="tmp")
                nc.vector.scalar_tensor_tensor(
                    out=tmp, in0=cent, scalar=rstd, in1=gw_sb,
                    op0=MULT, op1=MULT)
                yfin = small.tile([RPG, E], F32, tag="yfin")
                nc.vector.tensor_tensor(
                    out=yfin, in0=tmp, in1=bw_sb, op=ADD)
                nc.scalar.dma_start(out=yout.ap()[r0:r0 + RPG, :], in_=yfin)

    _split_excess_waits(nc)
    return nc


def _get_nc(mode, cc, pay):
    key = (mode, cc, pay, G)
    if key not in _NC_CACHE:
        if mode == "v2":
            _NC_CACHE[key] = _build_nc_v2(cc, pay)
        elif mode == "v3":
            _NC_CACHE[key] = _build_nc_v3(cc, pay)
        else:
            raise ValueError(f"unknown mode {mode}")
    return _NC_CACHE[key]


def _prefix_wt():
    idx = np.arange(S)
    diff = idx[:, None] - idx[None, :]          # i - j
    W = np.where(diff > 0, 1.0 / np.square(np.maximum(diff, 1)), 0.0)
    return np.ascontiguousarray(W.T.astype(np.float32))   # WT[j, i] = W[i, j]


def kernel(x, concept_map, gamma, beta, mode=None, trace=False):
    global LAST_RESULTS
    mode = mode or MODE
    import ml_dtypes  # noqa: F401  (registers bfloat16 with numpy)
    xf = np.ascontiguousarray(np.asarray(x, dtype=np.float32).reshape(T, E))
    cmap = np.asarray(concept_map, dtype=np.float32)
    gammaf = np.asarray(gamma, dtype=np.float32)
    betaf = np.asarray(beta, dtype=np.float32)

    wt_np = _prefix_wt().astype(ml_dtypes.bfloat16)
    # C_perm[a, p, c] = concept_map[c, a, p]
    cperm = np.ascontiguousarray(np.transpose(cmap, (1, 2, 0)))
    x_bf = xf.astype(ml_dtypes.bfloat16)

    TG = T // G
    # p-major packings shared across cores
    # xin[p, tc, c] = x[tc*128+p, c];  wt[p, j, i] = W^T[j*128+p, i]
    xin_p = np.ascontiguousarray(
        x_bf.reshape(TCH, 128, E).transpose(1, 0, 2))
    wt_p = np.ascontiguousarray(
        wt_np.reshape(S // 128, 128, S).transpose(1, 0, 2))
    gb = np.ascontiguousarray(np.broadcast_to(gammaf, (RPG, E))).astype(
        np.float32)
    bb = np.ascontiguousarray(np.broadcast_to(betaf, (RPG, E))).astype(
        np.float32)

    # ccin row r of group g holds token g*TG + (r%128)*TPG + r//128 (the
    # on-device transpose writes (p, tl)-major rows); core c owns rows
    # [SPG*c, SPG*(c+1)) of each group
    TPGh = TCH // G
    own_tok = np.empty((NCORES, G, SPG), dtype=np.int64)
    for c in range(NCORES):
        for g in range(G):
            r = SPG * c + np.arange(SPG)
            own_tok[c, g] = g * TG + (r % TPGh) * 128 + r // TPGh

    in_maps = []
    for c in range(NCORES):
        a0 = c * ASH
        own = np.stack([xf[own_tok[c, g]] for g in range(G)],
                       axis=1)  # [RPG, G, E]
        im = {
            "xin": xin_p,
            "wt": wt_p,
            "xres": np.ascontiguousarray(own),
            "gw": gb,
            "bw": bb,
            "ident": np.eye(128, dtype=ml_dtypes.bfloat16),
        }
        if mode == "v2":
            im["cs"] = np.ascontiguousarray(cperm[a0:a0 + ASH]).astype(
                ml_dtypes.bfloat16)
            im["xa"] = np.ascontiguousarray(xf[:, a0:a0 + ASH])
        else:
            # cs[pc, p, a, c] = C_perm[a0+a, pc*128+p, c]
            cslice = cperm[a0:a0 + ASH].astype(ml_dtypes.bfloat16)
            im["cs"] = np.ascontiguousarray(
                cslice.reshape(ASH, 2, 128, E).transpose(1, 2, 0, 3))
            # xbc[g, p, a, t] = x[g*TG + t, a0 + a]  (replicated over p)
            xs = x_bf[:, a0:a0 + ASH].reshape(G, TG, ASH).transpose(0, 2, 1)
            im["xbc"] = np.ascontiguousarray(
                np.broadcast_to(xs[:, None, :, :], (G, 128, ASH, TG)))
        in_maps.append(im)

    cc = os.environ.get("KCC", "a2a")
    pay = os.environ.get("KPAY", "bf16")
    nc = _get_nc(mode, cc, pay)
    res = None
    for attempt in range(4):
        try:
            res = run_bass_kernel_spmd(nc, in_maps, list(range(NCORES)),
                                       trace=trace)
            break
        except Exception:
            # transient NRT_EXEC_UNIT_UNRECOVERABLE happens occasionally on
            # the first dispatch after a fresh compile; back off and retry
            if attempt == 3:
                raise
            import time
            time.sleep(10 * (attempt + 1))
    LAST_RESULTS = res
    out = np.empty((T, E), dtype=np.float32)
    for c in range(NCORES):
        yc = res.results[c]["yout"]
        for g in range(G):
            out[own_tok[c, g]] = yc[g * RPG:(g + 1) * RPG]
    return np.ascontiguousarray(out.reshape(B, S, E).astype(np.float32))


# revision 53
# speedup vs baseline: 1.8865x; 1.0898x over previous
"""Trainium2 Bass kernel for nn_ConceptLayer (B=2, S=512, E=256), 8 NeuronCores.

Math:
  s[b,i,:] = sum_{j<i} x[b,j,:] / (i-j)^2            (prefix matmul, W constant)
  y[b,i,c] = sum_{a,p} x[b,i,a] * s[b,i,p] * C[c,a,p]
  out      = LayerNorm(y + x) * gamma + beta          (eps=1e-3)

Sharding: contraction axis `a` split 8 ways (32 a's per core); each core streams
1/8 of the concept_map (bf16, 4MB) once into SBUF and keeps it resident.

v2 device algorithm per core:
  phase 1: SmT[p,t] = s^T via PE (x bf16 stationary, W^T bf16 moving) -> bf16
  phase 2: token-group pipeline (G groups of T/G tokens):
      for each a-pair: psum[t,2*E] = SmT^T @ [C_a0|C_a1]  (bf16 matmul, N=512)
      DVE: y[t] (+)= x[t,a] * psum-half   (scalar_tensor_tensor, per-partition
           scalar; final a writes bf16 y_fin directly)
      after each group: DMA y_fin -> ccin_g, AllToAll (bf16, overlapped with the
      next group's matmuls), local 8-way reduce, +x residual, LayerNorm, store.
Host gathers the 8 [128,E] outputs and reorders token slices.
"""
import os
import numpy as np

import concourse.bass as bass
import concourse.mybir as mybir
import concourse.tile as tile
from concourse.bass_utils import run_bass_kernel_spmd

# ----------------------------------------------------------------------------
# constants (hardcoded per problem spec)
B, S, E = 2, 512, 256
T = B * S                      # 1024 tokens
NCORES = 8
ASH = E // NCORES              # 32 contraction-a values per core
TCH = T // 128                 # 8 token chunks of 128
LN_EPS = 1e-3

G = int(os.environ.get("KG", "2"))          # token groups (collective staging)
TPG = TCH // G                 # t-chunks per group
RPG = 128 // G                 # output rows per (core, group)
SPG = T // G // NCORES         # tokens per (core, group) == RPG

F32 = mybir.dt.float32
BF16 = mybir.dt.bfloat16
MULT = mybir.AluOpType.mult
ADD = mybir.AluOpType.add
AF = mybir.ActivationFunctionType

MODE = os.environ.get("KMODE", "v3")

LAST_RESULTS = None            # BassKernelResults of the last run (for test.py)

_NC_CACHE = {}


def _install_ntff_hook():
    """antenv.axon_hooks is absent in this image; recreate it so
    run_bass_kernel_spmd(trace=True) can drive NTFF profiling via the
    libaxon_pjrt.so C ABI (same recipe as trn_agent_boot)."""
    import sys, types, ctypes, contextlib  # noqa: E401

    if "antenv.axon_hooks" in sys.modules:
        return
    so_path = "/opt/axon/libaxon_pjrt.so"
    try:
        lib = ctypes.CDLL(so_path)
    except OSError:
        return
    if not hasattr(lib, "axon_start_nrt_profile"):
        return
    lib.axon_start_nrt_profile.argtypes = [
        ctypes.POINTER(ctypes.c_int64), ctypes.c_size_t]
    lib.axon_start_nrt_profile.restype = ctypes.c_int64
    lib.axon_stop_nrt_profile.argtypes = [ctypes.c_char_p]
    lib.axon_stop_nrt_profile.restype = ctypes.c_int64

    @contextlib.contextmanager
    def _hook(output_dir, device_ids):
        import jax
        jax.devices()
        if device_ids:
            ids = (ctypes.c_int64 * len(device_ids))(*device_ids)
            rc = lib.axon_start_nrt_profile(ids, len(device_ids))
        else:
            rc = lib.axon_start_nrt_profile(None, 0)
        if rc != 0:
            raise RuntimeError(f"axon_start_nrt_profile rc={rc}")
        try:
            yield
        finally:
            n = lib.axon_stop_nrt_profile(str(output_dir).encode())
            print(f"profile: {n} file(s) written to {output_dir}")

    mod = types.ModuleType("antenv.axon_hooks")
    mod.get_axon_ntff_profile_hook = lambda: _hook
    mod.set_axon_ntff_profile_hook = lambda h: None
    sys.modules["antenv.axon_hooks"] = mod


_install_ntff_hook()


def _split_excess_waits(nc):
    """walrus CoreV3 codegen allows only one sync-wait on Drain instructions;
    Tile's tail drain aggregates one wait per outstanding semaphore.  Move the
    excess onto NOPs inserted just before the offender (same engine)."""
    for fn in nc.m.functions:
        for bb in fn.blocks:
            insts = bb.instructions
            i = 0
            while i < len(insts):
                inst = insts[i]
                si = inst.sync_info
                max_waits = 1
                if si is not None and si.on_wait and len(si.on_wait) > max_waits:
                    waits = list(si.on_wait)
                    si.on_wait = waits[:max_waits]
                    extra = waits[max_waits:]
                    new_nops = []
                    for j in range(0, len(extra), max_waits):
                        nop = nc.engines[inst.engine].nop(nofuse=True).ins
                        nop.sync_info = mybir.SyncInfo(
                            on_wait=extra[j : j + max_waits], on_update=[]
                        )
                        new_nops.append(nop)
                    for nop in new_nops:
                        for fb in fn.blocks:
                            if nop in fb.instructions:
                                fb.instructions.remove(nop)
                    idx = insts.index(inst)
                    for k, nop in enumerate(new_nops):
                        insts.insert(idx + k, nop)
                    i = insts.index(inst)
                i += 1


def _build_nc_v2(cc="a2a", pay="bf16"):
    nc = bass.Bass("TRN2", target_bir_lowering=False, debug=False,
                   num_devices=NCORES)
    PAY = BF16 if pay == "bf16" else F32

    xin = nc.dram_tensor("xin", [T, E], BF16, kind="ExternalInput")
    xa = nc.dram_tensor("xa", [T, ASH], F32, kind="ExternalInput")
    wt = nc.dram_tensor("wt", [S, S], BF16, kind="ExternalInput")
    cs = nc.dram_tensor("cs", [ASH, E, E], BF16, kind="ExternalInput")
    xres = nc.dram_tensor("xres", [128, E], F32, kind="ExternalInput")
    gw = nc.dram_tensor("gw", [128, E], F32, kind="ExternalInput")
    bw = nc.dram_tensor("bw", [128, E], F32, kind="ExternalInput")
    yout = nc.dram_tensor("yout", [128, E], F32, kind="ExternalOutput")

    TG = T // G
    ccin = [nc.dram_tensor(f"ccin{g}", [TG, E], PAY) for g in range(G)]
    if cc == "a2a":
        ccout = [nc.dram_tensor(f"a2a{g}", [TG, E], PAY) for g in range(G)]
    else:
        ccout = [nc.dram_tensor(f"rs{g}", [SPG, E], PAY) for g in range(G)]

    with tile.TileContext(nc) as tc:
        import contextlib
        with contextlib.ExitStack() as ctx:
            consts = ctx.enter_context(tc.tile_pool(name="consts", bufs=1))
            small = ctx.enter_context(tc.tile_pool(name="small", bufs=2))

            # ---------------- phase 0: async constant loads ------------------
            # two HWDGE rings (sync=SP, scalar=ACT); alternate the big C parts
            x_all = consts.tile([128, TCH, E], BF16, tag="x_all")
            nc.sync.dma_start(
                out=x_all, in_=xin.ap().rearrange("(tc p) c -> p tc c", p=128))
            wt_all = consts.tile([128, S // 128, S], BF16, tag="wt_all")
            nc.scalar.dma_start(
                out=wt_all, in_=wt.ap().rearrange("(j p) i -> p j i", p=128))
            xa_all = consts.tile([128, TCH, ASH], F32, tag="xa_all")
            nc.sync.dma_start(
                out=xa_all, in_=xa.ap().rearrange("(tc p) a -> p tc a", p=128))
            # resident concept-map slice: c_res[:, pc, a, :] = C[a, 128pc:, c]
            c_res = consts.tile([128, 2, ASH, E], BF16, tag="c_res")
            NCH = 4                       # 4 a-ranges x 2 pc = 8 x 512KB DMAs
            ACH = ASH // NCH
            for i in range(NCH):
                for pc in range(2):
                    src = cs.ap()[i * ACH:(i + 1) * ACH,
                                  pc * 128:(pc + 1) * 128, :]
                    eng = nc.sync if (2 * i + pc) % 2 == 0 else nc.scalar
                    eng.dma_start(
                        out=c_res[:, pc, i * ACH:(i + 1) * ACH, :],
                        in_=src.rearrange("a p c -> p a c"))
            xres_sb = consts.tile([RPG, G, E], F32, tag="xres")
            nc.scalar.dma_start(
                out=xres_sb, in_=xres.ap().rearrange("(g p) c -> p g c", p=RPG))
            gw_sb = consts.tile([RPG, E], F32, tag="gw")
            nc.scalar.dma_start(out=gw_sb, in_=gw.ap()[0:RPG, :])
            bw_sb = consts.tile([RPG, E], F32, tag="bw")
            nc.scalar.dma_start(out=bw_sb, in_=bw.ap()[0:RPG, :])

            # ---------------- phase 1: SmT = (W @ x)^T -----------------------
            # SmT[d, t] per batch: lhsT = x[b] j-chunk (j, d-slice), rhs = W^T
            smT = [consts.tile([128, T], BF16, tag=f"smT{d}", name=f"smT{d}")
                   for d in range(E // 128)]
            with tc.tile_pool(name="ps_s", bufs=2, space="PSUM") as ps_s:
                for b in range(B):
                    for d in range(E // 128):
                        ps = ps_s.tile([128, S], F32, tag="ps_s")
                        for j in range(S // 128):
                            nc.tensor.matmul(
                                ps,
                                lhsT=x_all[:, b * 4 + j, d * 128:(d + 1) * 128],
                                rhs=wt_all[:, j, :],
                                start=(j == 0),
                                stop=(j == S // 128 - 1),
                            )
                        nc.scalar.copy(smT[d][:, b * S:(b + 1) * S], ps)

            # ---------------- phase 2: grouped contraction -------------------
            ps_y = ctx.enter_context(
                tc.tile_pool(name="ps_y", bufs=4, space="PSUM"))
            y_acc = [consts.tile([128, E], F32, tag=f"yacc{t}",
                                 name=f"yacc{t}")
                     for t in range(TCH)]
            y_fin = consts.tile([128, TPG, E], PAY, tag="y_fin")

            for g in range(G):
                for ap_i in range(ASH // 2):          # a-pairs, C resident
                    for tl in range(TPG):
                        t = g * TPG + tl
                        ps = ps_y.tile([128, 2 * E], F32, tag="ps_y")
                        for pc in range(2):
                            nc.tensor.matmul(
                                ps,
                                lhsT=smT[pc][:, t * 128:(t + 1) * 128],
                                rhs=c_res[:, pc, 2 * ap_i:2 * ap_i + 2, :]
                                    .rearrange("p a c -> p (a c)"),
                                start=(pc == 0),
                                stop=(pc == 1),
                            )
                        for ai in range(2):
                            a = 2 * ap_i + ai
                            half = ps[:, ai * E:(ai + 1) * E]
                            xs = xa_all[:, t, a:a + 1]
                            if a == 0:
                                nc.vector.tensor_scalar(
                                    out=y_acc[t], in0=half, scalar1=xs,
                                    scalar2=None, op0=MULT)
                            elif a == ASH - 1:
                                nc.vector.scalar_tensor_tensor(
                                    out=y_fin[:, tl, :], in0=half, scalar=xs,
                                    in1=y_acc[t], op0=MULT, op1=ADD)
                            else:
                                nc.vector.scalar_tensor_tensor(
                                    out=y_acc[t], in0=half, scalar=xs,
                                    in1=y_acc[t], op0=MULT, op1=ADD)
                # ship group g partials, start collective
                nc.sync.dma_start(
                    out=ccin[g].ap().rearrange("(tc p) c -> p tc c", p=128),
                    in_=y_fin)
                nc.gpsimd.collective_compute(
                    "AllToAll" if cc == "a2a" else "ReduceScatter",
                    mybir.AluOpType.bypass if cc == "a2a" else ADD,
                    replica_groups=[list(range(NCORES))],
                    ins=[ccin[g].ap()],
                    outs=[ccout[g].ap()],
                )

            # ---------------- phase 3: per-group reduce + LN -----------------
            for g in range(G):
                if cc == "a2a":
                    gb = small.tile([RPG, NCORES, E], PAY, tag="gbuf")
                    nc.sync.dma_start(
                        out=gb,
                        in_=ccout[g].ap().rearrange("(s p) c -> p s c", p=RPG))
                    g4 = small.tile([RPG, 4, E], F32, tag="g4")
                    nc.vector.tensor_tensor(
                        out=g4, in0=gb[:, 0:4, :], in1=gb[:, 4:8, :], op=ADD)
                    g2 = small.tile([RPG, 2, E], F32, tag="g2")
                    nc.vector.tensor_tensor(
                        out=g2, in0=g4[:, 0:2, :], in1=g4[:, 2:4, :], op=ADD)
                    yred = small.tile([RPG, E], F32, tag="yred")
                    nc.vector.tensor_tensor(
                        out=yred, in0=g2[:, 0, :], in1=g2[:, 1, :], op=ADD)
                else:
                    yred = small.tile([RPG, E], F32, tag="yred")
                    nc.sync.dma_start(out=yred, in_=ccout[g].ap())
                r0 = g * RPG
                nc.vector.tensor_tensor(
                    out=yred, in0=yred, in1=xres_sb[:, g, :], op=ADD)
                stats = small.tile([RPG, 6], F32, tag="stats")
                nc.vector.bn_stats(out=stats, in_=yred)
                mv = small.tile([RPG, 2], F32, tag="mv")
                nc.vector.bn_aggr(out=mv, in_=stats)
                eps_t = small.tile([RPG, 1], F32, tag="eps")
                nc.vector.memset(eps_t, LN_EPS)
                std = small.tile([RPG, 1], F32, tag="std")
                nc.scalar.activation(out=std, in_=mv[:, 1:2], func=AF.Sqrt,
                                     bias=eps_t)
                rstd = small.tile([RPG, 1], F32, tag="rstd")
                nc.vector.reciprocal(out=rstd, in_=std)
                cent = small.tile([RPG, E], F32, tag="cent")
                nc.vector.tensor_scalar(
                    out=cent, in0=yred, scalar1=mv[:, 0:1], scalar2=None,
                    op0=mybir.AluOpType.subtract)
                tmp = small.tile([RPG, E], F32, tag="tmp")
                nc.vector.scalar_tensor_tensor(
                    out=tmp, in0=cent, scalar=rstd, in1=gw_sb,
                    op0=MULT, op1=MULT)
                yfin = small.tile([RPG, E], F32, tag="yfin")
                nc.vector.tensor_tensor(
                    out=yfin, in0=tmp, in1=bw_sb, op=ADD)
                nc.sync.dma_start(out=yout.ap()[r0:r0 + RPG, :], in_=yfin)

    _split_excess_waits(nc)
    return nc


def _build_nc_v3(cc="a2a", pay="bf16"):
    """Transposed phase 2: stationary = concept-map chunks, moving = x-scaled
    s^T; psum accumulates y^T over the whole (a, p) sweep per token group."""
    nc = bass.Bass("TRN2", target_bir_lowering=False, debug=False,
                   num_devices=NCORES)
    PAY = BF16 if pay == "bf16" else F32
    TG = T // G

    # all inputs in p-major layouts: per-partition data is one contiguous
    # run, so DMA descriptors are 4-32KB (small descriptors gut DMA rate)
    xin = nc.dram_tensor("xin", [128, TCH, E], BF16, kind="ExternalInput")
    xbc = nc.dram_tensor("xbc", [G, 128, ASH, T // G], BF16,
                         kind="ExternalInput")
    wt = nc.dram_tensor("wt", [128, S // 128, S], BF16, kind="ExternalInput")
    cs = nc.dram_tensor("cs", [2, 128, ASH, E], BF16, kind="ExternalInput")
    xres = nc.dram_tensor("xres", [RPG, G, E], F32, kind="ExternalInput")
    gw = nc.dram_tensor("gw", [RPG, E], F32, kind="ExternalInput")
    bw = nc.dram_tensor("bw", [RPG, E], F32, kind="ExternalInput")
    ident = nc.dram_tensor("ident", [128, 128], BF16, kind="ExternalInput")
    yout = nc.dram_tensor("yout", [128, E], F32, kind="ExternalOutput")

    ccin = [nc.dram_tensor(f"ccin{g}", [TG, E], PAY) for g in range(G)]
    if cc == "a2a":
        ccout = [nc.dram_tensor(f"a2a{g}", [TG, E], PAY) for g in range(G)]
    else:
        ccout = [nc.dram_tensor(f"rs{g}", [SPG, E], PAY) for g in range(G)]

    with tile.TileContext(nc) as tc:
        import contextlib
        with contextlib.ExitStack() as ctx:
            consts = ctx.enter_context(tc.tile_pool(name="consts", bufs=1))
            small = ctx.enter_context(tc.tile_pool(name="small", bufs=2))
            scp = ctx.enter_context(tc.tile_pool(name="scp", bufs=4))

            # ---------------- phase 0: async constant loads ------------------
            x_all = consts.tile([128, TCH, E], BF16, tag="x_all")
            nc.sync.dma_start(out=x_all, in_=xin.ap())
            wt_all = consts.tile([128, S // 128, S], BF16, tag="wt_all")
            nc.scalar.dma_start(out=wt_all, in_=wt.ap())
            xres_sb = consts.tile([RPG, G, E], F32, tag="xres")
            nc.scalar.dma_start(out=xres_sb, in_=xres.ap())
            gw_sb = consts.tile([RPG, E], F32, tag="gw")
            nc.scalar.dma_start(out=gw_sb, in_=gw.ap())
            bw_sb = consts.tile([RPG, E], F32, tag="bw")
            nc.scalar.dma_start(out=bw_sb, in_=bw.ap())
            idn = consts.tile([128, 128], BF16, tag="idn")
            nc.scalar.dma_start(out=idn, in_=ident.ap())

            # resident concept-map slice + host-replicated x-broadcast rows,
            # a-chunked DMAs issued in consumption order:
            #   c_res[:, pc, a, :] = C[a0+a, 128pc+p, c]
            #   bc_g[g][:, a, t]   = x[g*TG + t, a0+a]
            NCH = 4
            ACH = ASH // NCH
            c_ch = []
            bc_ch = []
            for i in range(NCH):
                a0 = i * ACH
                ct = consts.tile([128, 2, ACH, E], BF16, tag=f"cch{i}",
                                 name=f"cch{i}")
                for pc in range(2):
                    eng = nc.sync if pc == 0 else nc.scalar
                    eng.dma_start(
                        out=ct[:, pc, :, :],
                        in_=cs.ap()[pc, :, a0:a0 + ACH, :])
                c_ch.append(ct)
                bt = consts.tile([128, ACH, TG], BF16, tag=f"bc{i}_0",
                                 name=f"bc{i}_0")
                eng = nc.sync if i % 2 == 0 else nc.scalar
                eng.dma_start(out=bt, in_=xbc.ap()[0, :, a0:a0 + ACH, :])
                bc_ch.append([bt])
            # group >0 broadcast rows can land during the group-0 sweep
            for g in range(1, G):
                for i in range(NCH):
                    a0 = i * ACH
                    bt = consts.tile([128, ACH, TG], BF16, tag=f"bc{i}_{g}",
                                     name=f"bc{i}_{g}")
                    eng = nc.sync if i % 2 == 0 else nc.scalar
                    eng.dma_start(out=bt, in_=xbc.ap()[g, :, a0:a0 + ACH, :])
                    bc_ch[i].append(bt)

            # ---------------- phase 1: SmT = (W @ x)^T -----------------------
            smT2 = consts.tile([128, 2, T], BF16, tag="smT2")
            with tc.tile_pool(name="ps_s", bufs=2, space="PSUM") as ps_s:
                for b in range(B):
                    for d in range(E // 128):
                        ps = ps_s.tile([128, S], F32, tag="ps_s")
                        tri = os.environ.get("KTRI", "1") == "1"
                        for j in range(S // 128):
                            # W^T[j, i] == 0 for i <= 128*j (lower-triangular)
                            i0 = 128 * j if (tri and j > 0) else 0
                            nc.tensor.matmul(
                                ps[:, i0:] if i0 else ps,
                                lhsT=x_all[:, b * 4 + j, d * 128:(d + 1) * 128],
                                rhs=wt_all[:, j, i0:] if i0 else wt_all[:, j, :],
                                start=(j == 0),
                                stop=(j == S // 128 - 1),
                            )
                        nc.scalar.copy(smT2[:, d, b * S:(b + 1) * S], ps)

            # ---------------- phase 2: transposed contraction ----------------
            ps_y = ctx.enter_context(
                tc.tile_pool(name="ps_y", bufs=2, space="PSUM"))
            y_fin = consts.tile([128, TPG, E], PAY, tag="y_fin")

            for g in range(G):
                t0 = g * TG
                psum_yT = [ps_y.tile([128, TG], F32, tag=f"psyT{cc_}",
                                     name=f"psyT{g}_{cc_}")
                           for cc_ in range(2)]
                for a in range(ASH):
                    # one DVE op builds both p-chunks of the scaled s^T:
                    # sc2[:, pc, t] = smT2[:, pc, t0+t] * x[t0+t, a]
                    # (bc row broadcast across pc via a stride-0 middle dim)
                    sc2 = scp.tile([128, 2, TG], BF16, tag="sc2",
                                   name=f"sc{g}_{a}")
                    row = bc_ch[a // ACH][g][:, a % ACH, :]
                    bcast = bass.AP(tensor=row.tensor, offset=row.offset,
                                    ap=[list(row.ap[0]), [0, 2],
                                        list(row.ap[1])])
                    nc.vector.tensor_tensor(
                        out=sc2, in0=smT2[:, :, t0:t0 + TG],
                        in1=bcast, op=MULT)
                    for cc_ in range(2):
                        for pc in range(2):
                            nc.tensor.matmul(
                                psum_yT[cc_],
                                lhsT=c_ch[a // ACH][:, pc, a % ACH,
                                                    cc_ * 128:(cc_ + 1) * 128],
                                rhs=sc2[:, pc, :],
                                start=(a == 0 and pc == 0),
                                stop=(a == ASH - 1 and pc == 1),
                            )
                # drain y^T (bf16), transpose 128x128 blocks on the PE:
                # y_fin[p, tl, c] = y^T[c, t0 + tl*128 + p]
                yT = []
                for cc_ in range(2):
                    y_t = small.tile([128, TG], BF16, tag=f"yT{cc_}",
                                     name=f"yT{g}_{cc_}")
                    nc.scalar.copy(y_t, psum_yT[cc_])
                    yT.append(y_t)
                with tc.tile_pool(name=f"pxp{g}", bufs=2,
                                  space="PSUM") as pxp:
                    for tl in range(TPG):
                        for cc_ in range(2):
                            pt = pxp.tile([128, 128], BF16, tag="pxp")
                            nc.tensor.transpose(
                                pt, in_=yT[cc_][:, tl * 128:(tl + 1) * 128],
                                identity=idn)
                            nc.scalar.copy(
                                y_fin[:, tl, cc_ * 128:(cc_ + 1) * 128], pt)
                # ccin linear row l = p*TPG + tc holds token t0 + tc*128 + p
                nc.sync.dma_start(
                    out=ccin[g].ap().rearrange("(p tc) c -> p tc c", p=128),
                    in_=y_fin)
                nc.gpsimd.collective_compute(
                    "AllToAll" if cc == "a2a" else "ReduceScatter",
                    mybir.AluOpType.bypass if cc == "a2a" else ADD,
                    replica_groups=[list(range(NCORES))],
                    ins=[ccin[g].ap()],
                    outs=[ccout[g].ap()],
                )

            # ---------------- phase 3: per-group reduce + LN -----------------
            for g in range(G):
                if cc == "a2a":
                    gb = small.tile([RPG, NCORES, E], PAY, tag="gbuf")
                    nc.gpsimd.dma_start(
                        out=gb,
                        in_=ccout[g].ap().rearrange("(s p) c -> p s c", p=RPG))
                    g4 = small.tile([RPG, 4, E], F32, tag="g4")
                    nc.vector.tensor_tensor(
                        out=g4, in0=gb[:, 0:4, :], in1=gb[:, 4:8, :], op=ADD)
                    g2 = small.tile([RPG, 2, E], F32, tag="g2")
                    nc.vector.tensor_tensor(
                        out=g2, in0=g4[:, 0:2, :], in1=g4[:, 2:4, :], op=ADD)
                    yred = small.tile([RPG, E], F32, tag="yred")
                    nc.vector.tensor_tensor(
                        out=yred, in0=g2[:, 0, :], in1=g2[:, 1, :], op=ADD)
                else:
                    yred = small.tile([RPG, E], F32, tag="yred")
                    nc.gpsimd.dma_start(out=yred, in_=ccout[g].ap())
                r0 = g * RPG
                nc.vector.tensor_tensor(
                    out=yred, in0=yred, in1=xres_sb[:, g, :], op=ADD)
                stats = small.tile([RPG, 6], F32, tag="stats")
                nc.vector.bn_stats(out=stats, in_=yred)
                mv = small.tile([RPG, 2], F32, tag="mv")
                nc.vector.bn_aggr(out=mv, in_=stats)
                eps_t = small.tile([RPG, 1], F32, tag="eps")
                nc.vector.memset(eps_t, LN_EPS)
                std = small.tile([RPG, 1], F32, tag="std")
                nc.scalar.activation(out=std, in_=mv[:, 1:2], func=AF.Sqrt,
                                     bias=eps_t)
                rstd = small.tile([RPG, 1], F32, tag="rstd")
                nc.vector.reciprocal(out=rstd, in_=std)
                cent = small.tile([RPG, E], F32, tag="cent")
                nc.vector.tensor_scalar(
                    out=cent, in0=yred, scalar1=mv[:, 0:1], scalar2=None,
                    op0=mybir.AluOpType.subtract)
                tmp = small.tile([RPG, E], F32, tag="tmp")
                nc.vector.scalar_tensor_tensor(
                    out=tmp, in0=cent, scalar=rstd, in1=gw_sb,
                    op0=MULT, op1=MULT)
                yfin = small.tile([RPG, E], F32, tag="yfin")
                nc.vector.tensor_tensor(
                    out=yfin, in0=tmp, in1=bw_sb, op=ADD)
                nc.scalar.dma_start(out=yout.ap()[r0:r0 + RPG, :], in_=yfin)

    _split_excess_waits(nc)
    return nc


def _get_nc(mode, cc, pay):
    key = (mode, cc, pay, G)
    if key not in _NC_CACHE:
        if mode == "v2":
            _NC_CACHE[key] = _build_nc_v2(cc, pay)
        elif mode == "v3":
            _NC_CACHE[key] = _build_nc_v3(cc, pay)
        else:
            raise ValueError(f"unknown mode {mode}")
    return _NC_CACHE[key]


def _prefix_wt():
    idx = np.arange(S)
    diff = idx[:, None] - idx[None, :]          # i - j
    W = np.where(diff > 0, 1.0 / np.square(np.maximum(diff, 1)), 0.0)
    return np.ascontiguousarray(W.T.astype(np.float32))   # WT[j, i] = W[i, j]


def kernel(x, concept_map, gamma, beta, mode=None, trace=False):
    global LAST_RESULTS
    mode = mode or MODE
    import ml_dtypes  # noqa: F401  (registers bfloat16 with numpy)
    xf = np.ascontiguousarray(np.asarray(x, dtype=np.float32).reshape(T, E))
    cmap = np.asarray(concept_map, dtype=np.float32)
    gammaf = np.asarray(gamma, dtype=np.float32)
    betaf = np.asarray(beta, dtype=np.float32)

    wt_np = _prefix_wt().astype(ml_dtypes.bfloat16)
    # C_perm[a, p, c] = concept_map[c, a, p]
    cperm = np.ascontiguousarray(np.transpose(cmap, (1, 2, 0)))
    x_bf = xf.astype(ml_dtypes.bfloat16)

    TG = T // G
    # p-major packings shared across cores
    # xin[p, tc, c] = x[tc*128+p, c];  wt[p, j, i] = W^T[j*128+p, i]
    xin_p = np.ascontiguousarray(
        x_bf.reshape(TCH, 128, E).transpose(1, 0, 2))
    wt_p = np.ascontiguousarray(
        wt_np.reshape(S // 128, 128, S).transpose(1, 0, 2))
    gb = np.ascontiguousarray(np.broadcast_to(gammaf, (RPG, E))).astype(
        np.float32)
    bb = np.ascontiguousarray(np.broadcast_to(betaf, (RPG, E))).astype(
        np.float32)

    # ccin row r of group g holds token g*TG + (r%128)*TPG + r//128 (the
    # on-device transpose writes (p, tl)-major rows); core c owns rows
    # [SPG*c, SPG*(c+1)) of each group
    TPGh = TCH // G
    own_tok = np.empty((NCORES, G, SPG), dtype=np.int64)
    for c in range(NCORES):
        for g in range(G):
            r = SPG * c + np.arange(SPG)
            own_tok[c, g] = g * TG + (r % TPGh) * 128 + r // TPGh

    in_maps = []
    for c in range(NCORES):
        a0 = c * ASH
        own = np.stack([xf[own_tok[c, g]] for g in range(G)],
                       axis=1)  # [RPG, G, E]
        im = {
            "xin": xin_p,
            "wt": wt_p,
            "xres": np.ascontiguousarray(own),
            "gw": gb,
            "bw": bb,
            "ident": np.eye(128, dtype=ml_dtypes.bfloat16),
        }
        if mode == "v2":
            im["cs"] = np.ascontiguousarray(cperm[a0:a0 + ASH]).astype(
                ml_dtypes.bfloat16)
            im["xa"] = np.ascontiguousarray(xf[:, a0:a0 + ASH])
        else:
            # cs[pc, p, a, c] = C_perm[a0+a, pc*128+p, c]
            cslice = cperm[a0:a0 + ASH].astype(ml_dtypes.bfloat16)
            im["cs"] = np.ascontiguousarray(
                cslice.reshape(ASH, 2, 128, E).transpose(1, 2, 0, 3))
            # xbc[g, p, a, t] = x[g*TG + t, a0 + a]  (replicated over p)
            xs = x_bf[:, a0:a0 + ASH].reshape(G, TG, ASH).transpose(0, 2, 1)
            im["xbc"] = np.ascontiguousarray(
                np.broadcast_to(xs[:, None, :, :], (G, 128, ASH, TG)))
        in_maps.append(im)

    cc = os.environ.get("KCC", "rs")
    pay = os.environ.get("KPAY", "bf16")
    nc = _get_nc(mode, cc, pay)
    res = None
    for attempt in range(4):
        try:
            res = run_bass_kernel_spmd(nc, in_maps, list(range(NCORES)),
                                       trace=trace)
            break
        except Exception:
            # transient NRT_EXEC_UNIT_UNRECOVERABLE happens occasionally on
            # the first dispatch after a fresh compile; back off and retry
            if attempt == 3:
                raise
            import time
            time.sleep(10 * (attempt + 1))
    LAST_RESULTS = res
    out = np.empty((T, E), dtype=np.float32)
    for c in range(NCORES):
        yc = res.results[c]["yout"]
        for g in range(G):
            out[own_tok[c, g]] = yc[g * RPG:(g + 1) * RPG]
    return np.ascontiguousarray(out.reshape(B, S, E).astype(np.float32))
